# revision 15
# baseline (speedup 1.0000x reference)
"""VQ codebook encoding (nn_Encoding) Trainium2 Bass kernel.

Math (per batch b):
    xf = x[b].reshape(C, N).T                      # (N, C)
    logits[n,k] = scale_k * (||xf_n||^2 - 2 xf_n.cw_k + ||cw_k||^2)
    w = softmax(logits, axis=k)
    enc[k,:]  = sum_n w[n,k] * (xf_n - cw_k)

Device decomposition (data-parallel over batch, 2 batches/core on 8 cores):
    - x^2 (ACT):  one Square op per 512-pixel segment, natural [c, n] layout.
    - mm1 (PE):   logits^T[k, n] = AT^T @ x + scalebc^T @ x^2, with
                  AT[c,k] = -2 scale_k cw[k,c] and scalebc[c,k] = scale_k
                  (the sum over c of scale_k x^2 rides the matmul), plus an
                  exp-bias scale_k*c2_k applied by the ACT Exp below. The
                  logits stay un-maxed: they are all <= 0 for this problem
                  family and exp underflow is harmless.
    - transposes (PE): x chunks [c,n] -> xT [n,c] via identity matmul (PSUM),
                  copied to SBUF on DVE/ACT.
    - softmax (ACT): numer = Exp(logits + bias) in bf16 (only the aggregation
                  weights pass through bf16; xT stays at f32r precision).
                  numer^T transposes (PE) -> per-n denominator via the
                  accum_out of the PSUM->SBUF copy (DVE tensor_scalar).
    - mm2 (PE):   enc[k,c] = sum_n w^T[n,k]^T @ xT[n,c], weights scaled by
                  1/denom on DVE; wsum via a -1s rhs column; final
                  enc += wsum*cw on DVE (scalar_tensor_tensor).
Matmuls run in float32r (full-rate 4-byte mode, ~1e-4 relative rounding).
End-to-end: ~99 us on hardware, relative error ~1.5e-4 vs the fp32 reference.
"""
import os
import numpy as np

B, C, N, K = 16, 512, 4096, 32
NCORES = 8
BPC = B // NCORES          # batches per core
SEG = 512                  # n per segment
NSEG = N // SEG
CC = C // 128              # c chunks
NB = SEG // 128            # n chunks per segment

_CACHE = {}


def _patch_tile_drain(tile, mybir, ScopedClock):
    """This walrus build rejects any instruction carrying >1 sync wait.
    Split extra waits onto single-wait NoOps on the same engine."""
    if getattr(tile.TileContext, "_multiwait_patched", False):
        return
    tile.TileContext._multiwait_patched = True

    _orig_add = tile.TileContext._add_instruction

    def _split_add(self, inst):
        si = inst.sync_info
        if si is not None and si.on_wait and len(si.on_wait) > 1:
            waits = list(si.on_wait)
            for w in waits[:-1]:
                nop = mybir.InstNoOp(name=f"waitnop-{self.nc.next_id()}", ins=[], outs=[])
                nop.engine = inst.engine
                nop.sync_info = mybir.SyncInfo(on_wait=[w], on_update=[])
                _orig_add(self, nop)
            si.on_wait = [waits[-1]]
            inst.sync_info = si
        _orig_add(self, inst)

    tile.TileContext._add_instruction = _split_add

    def _patched_drain(self, tick_clock, wait_clock):
        nc = self.nc
        probe = nc.sync.drain()
        wait_clock.add_sem_waits(probe.ins, ScopedClock({None: tick_clock.global_clock}))
        raw = probe.ins
        waits = list(raw.sync_info.on_wait) if raw.sync_info and raw.sync_info.on_wait else []
        if raw.sync_info is not None:
            raw.sync_info.on_wait = []
        for w in waits:
            wi = nc.sync.nop()
            wi.ins.sync_info = mybir.SyncInfo(on_wait=[w], on_update=[])
        nc.all_engine_barrier()
        assert self.sems is not None
        popped = nc._tile_sem_poison_stack.pop()
        assert popped is self._sem_poison
        nc.clear_and_free_semaphores(list(self.sems.allocated().values()))
        nc.all_engine_barrier()

    tile.TileContext._drain_and_barrier = _patched_drain


def _build():
    import concourse.bass as bass
    import concourse.tile as tile
    from concourse import mybir
    from concourse.vector_clock import ScopedClock

    _patch_tile_drain(tile, mybir, ScopedClock)

    F32 = mybir.dt.float32
    F32R = mybir.dt.float32r
    BF16 = mybir.dt.bfloat16
    Alu = mybir.AluOpType
    Act = mybir.ActivationFunctionType

    nc = bass.Bass("TRN2", target_bir_lowering=False, debug=False, num_devices=NCORES)
    x_ext = nc.dram_tensor("x", [BPC, C, N], F32R, kind="ExternalInput").ap()
    at_ext = nc.dram_tensor("at", [128, CC, K], F32R, kind="ExternalInput").ap()
    sbc_ext = nc.dram_tensor("sbc", [128, K], F32R, kind="ExternalInput").ap()
    bias_ext = nc.dram_tensor("bias", [K, 1], F32, kind="ExternalInput").ap()
    cw_ext = nc.dram_tensor("cw", [K, C], F32, kind="ExternalInput").ap()
    id_ext = nc.dram_tensor("ident", [128, 128], F32R, kind="ExternalInput").ap()
    id32_ext = nc.dram_tensor("ident32", [K, K], BF16, kind="ExternalInput").ap()
    enc_ext = nc.dram_tensor("enc", [BPC, K, C], F32, kind="ExternalOutput").ap()

    with tile.TileContext(nc) as tc:
        with (
            tc.tile_pool(name="singles", bufs=1) as singles,
            tc.tile_pool(name="xin", bufs=4) as xin,
            tc.tile_pool(name="xts", bufs=8) as xts,
            tc.tile_pool(name="small", bufs=2) as small,
            tc.tile_pool(name="outp", bufs=2) as outp,
            tc.tile_pool(name="ps_xt", bufs=4, space="PSUM") as ps_xt,
            tc.tile_pool(name="ps_lg", bufs=1, space="PSUM") as ps_lg,
            tc.tile_pool(name="ps_nt", bufs=1, space="PSUM") as ps_nt,
            tc.tile_pool(name="ps_enc", bufs=1, space="PSUM") as ps_enc,
            tc.tile_pool(name="ps_ws", bufs=1, space="PSUM") as ps_ws,
        ):
            id_sb = singles.tile([128, 128], F32R)
            nc.sync.dma_start(out=id_sb, in_=id_ext)
            at_sb = singles.tile([128, CC, K], F32R)
            nc.sync.dma_start(out=at_sb, in_=at_ext)
            sbc_sb = singles.tile([128, K], F32R)
            nc.sync.dma_start(out=sbc_sb, in_=sbc_ext)
            bias_sb = singles.tile([K, 1], F32)
            nc.sync.dma_start(out=bias_sb, in_=bias_ext)
            id32_sb = singles.tile([K, K], BF16)
            nc.sync.dma_start(out=id32_sb, in_=id32_ext)
            cw_sb = singles.tile([K, C], F32)
            nc.sync.dma_start(out=cw_sb, in_=cw_ext)
            negones_f = singles.tile([128, 8], F32)
            nc.vector.memset(negones_f, -1.0)
            negones = singles.tile([128, 8], F32R)
            nc.vector.tensor_copy(out=negones, in_=negones_f)

            for b in range(BPC):
                enc_ps = ps_enc.tile([K, C], F32, tag="enc")
                ws_ps = ps_ws.tile([K, 8], F32, tag="ws")
                for s in range(NSEG):
                    first = (s == 0)
                    # ---- load x segment: [p, cc, n512], c = cc*128 + p ----
                    x_sb = xin.tile([128, CC, SEG], F32R, tag="x")
                    x_src = x_ext[b].rearrange("(cc p) n -> p cc n", p=128)[
                        :, :, s * SEG:(s + 1) * SEG]
                    if b == 0 and s == 0:
                        # prime the pipeline: per-chunk DMAs let the first
                        # transposes start ~2us earlier
                        for cc in range(CC):
                            nc.sync.dma_start(out=x_sb[:, cc, :], in_=x_src[:, cc, :])
                    else:
                        nc.sync.dma_start(out=x_sb, in_=x_src)
                    # ---- x^2 in natural layout (one big ACT op); the sum over
                    # c and the scale_k factor ride mm1 on the PE below ----
                    x2sq = xin.tile([128, CC, SEG], F32R, tag="xsq")
                    nc.scalar.activation(
                        out=x2sq.rearrange("p cc n -> p (cc n)"),
                        in_=x_sb.rearrange("p cc n -> p (cc n)"),
                        func=Act.Square)
                    # ---- transposes + xT copies (f32r native) ----
                    xt_sbs = []
                    for nb in range(NB):
                        xt_ps = ps_xt.tile([128, C], F32R, tag="xt")
                        for cc in range(CC):
                            nc.tensor.transpose(
                                xt_ps[:, cc * 128:(cc + 1) * 128],
                                x_sb[:, cc, nb * 128:(nb + 1) * 128],
                                id_sb,
                            )
                        xt_sb = xts.tile([128, C], F32R, tag="xts")
                        if nb == 3:
                            nc.scalar.copy(out=xt_sb, in_=xt_ps)
                        else:
                            nc.vector.tensor_copy(out=xt_sb, in_=xt_ps)
                        xt_sbs.append(xt_sb)
                    # ---- mm1: logits^T [K, 512] = AT.T@x + scalebc.T@x^2 ----
                    lg_ps = ps_lg.tile([K, SEG], F32, tag="lg")
                    for cc in range(CC):
                        nc.tensor.matmul(lg_ps, at_sb[:, cc, :], x_sb[:, cc, :],
                                         start=(cc == 0), stop=False)
                    for cc in range(CC):
                        nc.tensor.matmul(lg_ps, sbc_sb, x2sq[:, cc, :],
                                         start=False, stop=(cc == CC - 1))
                    # ---- softmax numerator (bf16 is fine for the w side) ----
                    numer = small.tile([K, SEG], BF16, tag="numer")
                    nc.scalar.activation(out=numer, in_=lg_ps, func=Act.Exp,
                                         bias=bias_sb, scale=1.0)
                    # ---- numer transposes -> [n,k] + copy-with-denom-accum ----
                    nt_ps = ps_nt.tile([128, NB, K], BF16, tag="nt")
                    for nb in range(NB):
                        nc.tensor.transpose(
                            nt_ps[:, nb, :],
                            numer[:, nb * 128:(nb + 1) * 128],
                            id32_sb)
                    wt_sb = small.tile([128, NB, K], F32R, tag="wt")
                    dcols = small.tile([128, NB], F32, tag="dc")
                    for nb in range(NB):
                        nc.vector.tensor_scalar(
                            out=wt_sb[:, nb, :], in0=nt_ps[:, nb, :],
                            scalar1=1.0, scalar2=0.0,
                            op0=Alu.mult, op1=Alu.add,
                            accum_out=dcols[:, nb:nb + 1])
                    rden = small.tile([128, NB], F32, tag="rd")
                    nc.vector.reciprocal(rden, dcols)
                    for nb in range(NB):
                        nc.vector.tensor_scalar_mul(
                            wt_sb[:, nb, :], in0=wt_sb[:, nb, :],
                            scalar1=rden[:, nb:nb + 1])
                    # ---- mm2 + wsum ----
                    for nb in range(NB):
                        nc.tensor.matmul(enc_ps, wt_sb[:, nb, :], xt_sbs[nb],
                                         start=(first and nb == 0), stop=(s == NSEG - 1 and nb == NB - 1),
                                         skip_group_check=True)
                        nc.tensor.matmul(ws_ps, wt_sb[:, nb, :], negones,
                                         start=(first and nb == 0), stop=(s == NSEG - 1 and nb == NB - 1),
                                         skip_group_check=True)
                # ---- final: enc += wsum * cw (wsum is negative) ----
                enc_sb = outp.tile([K, C], F32, tag="enc_out")
                nc.vector.scalar_tensor_tensor(
                    out=enc_sb, in0=cw_sb, scalar=ws_ps[:, 0:1], in1=enc_ps,
                    op0=Alu.mult, op1=Alu.add)
                nc.sync.dma_start(out=enc_ext[b], in_=enc_sb)

    return nc


def kernel(x, codewords, scale):
    from concourse.bass_utils import run_bass_kernel_spmd

    x = np.ascontiguousarray(x, dtype=np.float32)
    codewords = np.ascontiguousarray(codewords, dtype=np.float32)
    scale = np.ascontiguousarray(scale, dtype=np.float32)

    if "nc" not in _CACHE:
        _CACHE["nc"] = _build()
    nc = _CACHE["nc"]

    # host-side tiny prep
    xr = x.reshape(B, C, N)
    at = (-2.0 * scale[:, None] * codewords).T.copy()          # [C, K]
    at = at.reshape(CC, 128, K).transpose(1, 0, 2).copy()      # [128, cc, K], c = cc*128+p
    c2 = (codewords.astype(np.float64) ** 2).sum(1).astype(np.float32)
    bias = (scale * c2).reshape(K, 1).astype(np.float32)
    sbc = np.broadcast_to(scale.reshape(1, K), (128, K)).astype(np.float32).copy()
    ident = np.eye(128, dtype=np.float32)
    import ml_dtypes
    ident32 = np.eye(K, dtype=ml_dtypes.bfloat16)

    in_maps = []
    for i in range(NCORES):
        in_maps.append({
            "x": np.ascontiguousarray(xr[i * BPC:(i + 1) * BPC]),
            "at": at, "sbc": sbc, "bias": bias,
            "cw": codewords, "ident": ident, "ident32": ident32,
        })
    tmpdir = os.environ.get("BASS_PROF_DIR") or None
    res = run_bass_kernel_spmd(nc, in_maps, list(range(NCORES)), tmpdir=tmpdir)
    _CACHE["last_results"] = res
    out = np.concatenate([res.results[i]["enc"] for i in range(NCORES)], axis=0)
    return out.astype(np.float32)


# revision 16
# speedup vs baseline: 1.0364x; 1.0364x over previous
"""VQ codebook encoding (nn_Encoding) Trainium2 Bass kernel.

Math (per batch b):
    xf = x[b].reshape(C, N).T                      # (N, C)
    logits[n,k] = scale_k * (||xf_n||^2 - 2 xf_n.cw_k + ||cw_k||^2)
    w = softmax(logits, axis=k)
    enc[k,:]  = sum_n w[n,k] * (xf_n - cw_k)

Device decomposition (data-parallel over batch, 2 batches/core on 8 cores):
    - x^2 (ACT):  one Square op per 512-pixel segment, natural [c, n] layout.
    - mm1 (PE):   logits^T[k, n] = AT^T @ x + scalebc^T @ x^2, with
                  AT[c,k] = -2 scale_k cw[k,c] and scalebc[c,k] = scale_k
                  (the sum over c of scale_k x^2 rides the matmul), plus an
                  exp-bias scale_k*c2_k applied by the ACT Exp below. The
                  logits stay un-maxed: they are all <= 0 for this problem
                  family and exp underflow is harmless.
    - transposes (PE): x chunks [c,n] -> xT [n,c] via identity matmul (PSUM),
                  copied to SBUF on DVE/ACT.
    - softmax (ACT): numer = Exp(logits + bias) in bf16 (only the aggregation
                  weights pass through bf16; xT stays at f32r precision).
                  numer^T transposes (PE) -> per-n denominator via the
                  accum_out of the PSUM->SBUF copy (DVE tensor_scalar).
    - mm2 (PE):   enc[k,c] = sum_n w^T[n,k]^T @ xT[n,c], weights scaled by
                  1/denom on DVE; wsum via a -1s rhs column; final
                  enc += wsum*cw on DVE (scalar_tensor_tensor).
Matmuls run in float32r (full-rate 4-byte mode, ~1e-4 relative rounding).
End-to-end: ~99 us on hardware, relative error ~1.5e-4 vs the fp32 reference.
"""
import os
import numpy as np

B, C, N, K = 16, 512, 4096, 32
NCORES = 8
BPC = B // NCORES          # batches per core
SEG = 512                  # n per segment
NSEG = N // SEG
CC = C // 128              # c chunks
NB = SEG // 128            # n chunks per segment

_CACHE = {}


def _patch_tile_drain(tile, mybir, ScopedClock):
    """This walrus build rejects any instruction carrying >1 sync wait.
    Split extra waits onto single-wait NoOps on the same engine."""
    if getattr(tile.TileContext, "_multiwait_patched", False):
        return
    tile.TileContext._multiwait_patched = True

    _orig_add = tile.TileContext._add_instruction

    def _split_add(self, inst):
        si = inst.sync_info
        if si is not None and si.on_wait and len(si.on_wait) > 1:
            waits = list(si.on_wait)
            for w in waits[:-1]:
                nop = mybir.InstNoOp(name=f"waitnop-{self.nc.next_id()}", ins=[], outs=[])
                nop.engine = inst.engine
                nop.sync_info = mybir.SyncInfo(on_wait=[w], on_update=[])
                _orig_add(self, nop)
            si.on_wait = [waits[-1]]
            inst.sync_info = si
        _orig_add(self, inst)

    tile.TileContext._add_instruction = _split_add

    def _patched_drain(self, tick_clock, wait_clock):
        nc = self.nc
        probe = nc.sync.drain()
        wait_clock.add_sem_waits(probe.ins, ScopedClock({None: tick_clock.global_clock}))
        raw = probe.ins
        waits = list(raw.sync_info.on_wait) if raw.sync_info and raw.sync_info.on_wait else []
        if raw.sync_info is not None:
            raw.sync_info.on_wait = []
        for w in waits:
            wi = nc.sync.nop()
            wi.ins.sync_info = mybir.SyncInfo(on_wait=[w], on_update=[])
        nc.all_engine_barrier()
        assert self.sems is not None
        popped = nc._tile_sem_poison_stack.pop()
        assert popped is self._sem_poison
        nc.clear_and_free_semaphores(list(self.sems.allocated().values()))
        nc.all_engine_barrier()

    tile.TileContext._drain_and_barrier = _patched_drain


def _build():
    import concourse.bass as bass
    import concourse.tile as tile
    from concourse import mybir
    from concourse.vector_clock import ScopedClock

    _patch_tile_drain(tile, mybir, ScopedClock)

    F32 = mybir.dt.float32
    F32R = mybir.dt.float32r
    BF16 = mybir.dt.bfloat16
    Alu = mybir.AluOpType
    Act = mybir.ActivationFunctionType

    nc = bass.Bass("TRN2", target_bir_lowering=False, debug=False, num_devices=NCORES)
    x_ext = nc.dram_tensor("x", [BPC, C, N], F32R, kind="ExternalInput").ap()
    at_ext = nc.dram_tensor("at", [128, CC, K], F32R, kind="ExternalInput").ap()
    sbc_ext = nc.dram_tensor("sbc", [128, K], F32R, kind="ExternalInput").ap()
    bias_ext = nc.dram_tensor("bias", [K, 1], F32, kind="ExternalInput").ap()
    cw_ext = nc.dram_tensor("cw", [K, C], F32, kind="ExternalInput").ap()
    id_ext = nc.dram_tensor("ident", [128, 128], F32R, kind="ExternalInput").ap()
    id32_ext = nc.dram_tensor("ident32", [K, K], BF16, kind="ExternalInput").ap()
    enc_ext = nc.dram_tensor("enc", [BPC, K, C], F32, kind="ExternalOutput").ap()

    with tile.TileContext(nc) as tc:
        with (
            tc.tile_pool(name="singles", bufs=1) as singles,
            tc.tile_pool(name="xin", bufs=4) as xin,
            tc.tile_pool(name="xts", bufs=8) as xts,
            tc.tile_pool(name="small", bufs=2) as small,
            tc.tile_pool(name="outp", bufs=2) as outp,
            tc.tile_pool(name="ps_xt", bufs=4, space="PSUM") as ps_xt,
            tc.tile_pool(name="ps_lg", bufs=1, space="PSUM") as ps_lg,
            tc.tile_pool(name="ps_nt", bufs=1, space="PSUM") as ps_nt,
            tc.tile_pool(name="ps_enc", bufs=1, space="PSUM") as ps_enc,
            tc.tile_pool(name="ps_ws", bufs=1, space="PSUM") as ps_ws,
        ):
            id_sb = singles.tile([128, 128], F32R)
            nc.gpsimd.dma_start(out=id_sb, in_=id_ext)
            at_sb = singles.tile([128, CC, K], F32R)
            nc.gpsimd.dma_start(out=at_sb, in_=at_ext)
            sbc_sb = singles.tile([128, K], F32R)
            nc.gpsimd.dma_start(out=sbc_sb, in_=sbc_ext)
            bias_sb = singles.tile([K, 1], F32)
            nc.gpsimd.dma_start(out=bias_sb, in_=bias_ext)
            id32_sb = singles.tile([K, K], BF16)
            nc.gpsimd.dma_start(out=id32_sb, in_=id32_ext)
            cw_sb = singles.tile([K, C], F32)
            nc.gpsimd.dma_start(out=cw_sb, in_=cw_ext)
            negones_f = singles.tile([128, 8], F32)
            nc.vector.memset(negones_f, -1.0)
            negones = singles.tile([128, 8], F32R)
            nc.vector.tensor_copy(out=negones, in_=negones_f)

            for b in range(BPC):
                enc_ps = ps_enc.tile([K, C], F32, tag="enc")
                ws_ps = ps_ws.tile([K, 8], F32, tag="ws")
                for s in range(NSEG):
                    first = (s == 0)
                    # ---- load x segment: [p, cc, n512], c = cc*128 + p ----
                    x_sb = xin.tile([128, CC, SEG], F32R, tag="x")
                    nc.sync.dma_start(
                        out=x_sb,
                        in_=x_ext[b].rearrange("(cc p) n -> p cc n", p=128)[
                            :, :, s * SEG:(s + 1) * SEG],
                    )
                    # ---- x^2 in natural layout (one big ACT op); the sum over
                    # c and the scale_k factor ride mm1 on the PE below ----
                    x2sq = xin.tile([128, CC, SEG], F32R, tag="xsq")
                    nc.scalar.activation(
                        out=x2sq.rearrange("p cc n -> p (cc n)"),
                        in_=x_sb.rearrange("p cc n -> p (cc n)"),
                        func=Act.Square)
                    # ---- transposes + xT copies (f32r native) ----
                    xt_sbs = []
                    for nb in range(NB):
                        xt_ps = ps_xt.tile([128, C], F32R, tag="xt")
                        for cc in range(CC):
                            nc.tensor.transpose(
                                xt_ps[:, cc * 128:(cc + 1) * 128],
                                x_sb[:, cc, nb * 128:(nb + 1) * 128],
                                id_sb,
                            )
                        xt_sb = xts.tile([128, C], F32R, tag="xts")
                        if nb == 3:
                            nc.scalar.copy(out=xt_sb, in_=xt_ps)
                        else:
                            nc.vector.tensor_copy(out=xt_sb, in_=xt_ps)
                        xt_sbs.append(xt_sb)
                    # ---- mm1: logits^T [K, 512] = AT.T@x + scalebc.T@x^2 ----
                    lg_ps = ps_lg.tile([K, SEG], F32, tag="lg")
                    for cc in range(CC):
                        nc.tensor.matmul(lg_ps, at_sb[:, cc, :], x_sb[:, cc, :],
                                         start=(cc == 0), stop=False)
                    for cc in range(CC):
                        nc.tensor.matmul(lg_ps, sbc_sb, x2sq[:, cc, :],
                                         start=False, stop=(cc == CC - 1))
                    # ---- softmax numerator (bf16 is fine for the w side) ----
                    numer = small.tile([K, SEG], BF16, tag="numer")
                    nc.scalar.activation(out=numer, in_=lg_ps, func=Act.Exp,
                                         bias=bias_sb, scale=1.0)
                    # ---- numer transposes -> [n,k] + copy-with-denom-accum ----
                    nt_ps = ps_nt.tile([128, NB, K], BF16, tag="nt")
                    for nb in range(NB):
                        nc.tensor.transpose(
                            nt_ps[:, nb, :],
                            numer[:, nb * 128:(nb + 1) * 128],
                            id32_sb)
                    wt_sb = small.tile([128, NB, K], F32R, tag="wt")
                    dcols = small.tile([128, NB], F32, tag="dc")
                    for nb in range(NB):
                        nc.vector.tensor_scalar(
                            out=wt_sb[:, nb, :], in0=nt_ps[:, nb, :],
                            scalar1=1.0, scalar2=0.0,
                            op0=Alu.mult, op1=Alu.add,
                            accum_out=dcols[:, nb:nb + 1])
                    rden = small.tile([128, NB], F32, tag="rd")
                    nc.vector.reciprocal(rden, dcols)
                    for nb in range(NB):
                        nc.vector.tensor_scalar_mul(
                            wt_sb[:, nb, :], in0=wt_sb[:, nb, :],
                            scalar1=rden[:, nb:nb + 1])
                    # ---- mm2 + wsum ----
                    for nb in range(NB):
                        nc.tensor.matmul(enc_ps, wt_sb[:, nb, :], xt_sbs[nb],
                                         start=(first and nb == 0), stop=(s == NSEG - 1 and nb == NB - 1),
                                         skip_group_check=True)
                        nc.tensor.matmul(ws_ps, wt_sb[:, nb, :], negones,
                                         start=(first and nb == 0), stop=(s == NSEG - 1 and nb == NB - 1),
                                         skip_group_check=True)
                # ---- final: enc += wsum * cw (wsum is negative) ----
                enc_sb = outp.tile([K, C], F32, tag="enc_out")
                nc.vector.scalar_tensor_tensor(
                    out=enc_sb, in0=cw_sb, scalar=ws_ps[:, 0:1], in1=enc_ps,
                    op0=Alu.mult, op1=Alu.add)
                nc.sync.dma_start(out=enc_ext[b], in_=enc_sb)

    return nc


def kernel(x, codewords, scale):
    from concourse.bass_utils import run_bass_kernel_spmd

    x = np.ascontiguousarray(x, dtype=np.float32)
    codewords = np.ascontiguousarray(codewords, dtype=np.float32)
    scale = np.ascontiguousarray(scale, dtype=np.float32)

    if "nc" not in _CACHE:
        _CACHE["nc"] = _build()
    nc = _CACHE["nc"]

    # host-side tiny prep
    xr = x.reshape(B, C, N)
    at = (-2.0 * scale[:, None] * codewords).T.copy()          # [C, K]
    at = at.reshape(CC, 128, K).transpose(1, 0, 2).copy()      # [128, cc, K], c = cc*128+p
    c2 = (codewords.astype(np.float64) ** 2).sum(1).astype(np.float32)
    bias = (scale * c2).reshape(K, 1).astype(np.float32)
    sbc = np.broadcast_to(scale.reshape(1, K), (128, K)).astype(np.float32).copy()
    ident = np.eye(128, dtype=np.float32)
    import ml_dtypes
    ident32 = np.eye(K, dtype=ml_dtypes.bfloat16)

    in_maps = []
    for i in range(NCORES):
        in_maps.append({
            "x": np.ascontiguousarray(xr[i * BPC:(i + 1) * BPC]),
            "at": at, "sbc": sbc, "bias": bias,
            "cw": codewords, "ident": ident, "ident32": ident32,
        })
    tmpdir = os.environ.get("BASS_PROF_DIR") or None
    res = run_bass_kernel_spmd(nc, in_maps, list(range(NCORES)), tmpdir=tmpdir)
    _CACHE["last_results"] = res
    out = np.concatenate([res.results[i]["enc"] for i in range(NCORES)], axis=0)
    return out.astype(np.float32)


# revision 17
# speedup vs baseline: 1.0848x; 1.0467x over previous
"""VQ codebook encoding (nn_Encoding) Trainium2 Bass kernel.

Math (per batch b):
    xf = x[b].reshape(C, N).T                      # (N, C)
    logits[n,k] = scale_k * (||xf_n||^2 - 2 xf_n.cw_k + ||cw_k||^2)
    w = softmax(logits, axis=k)
    enc[k,:]  = sum_n w[n,k] * (xf_n - cw_k)

Device decomposition (data-parallel over batch, 2 batches/core on 8 cores):
    - x^2 (ACT):  one Square op per 512-pixel segment, natural [c, n] layout.
    - mm1 (PE):   logits^T[k, n] = AT^T @ x + scalebc^T @ x^2, with
                  AT[c,k] = -2 scale_k cw[k,c] and scalebc[c,k] = scale_k
                  (the sum over c of scale_k x^2 rides the matmul), plus an
                  exp-bias scale_k*c2_k applied by the ACT Exp below. The
                  logits stay un-maxed: they are all <= 0 for this problem
                  family and exp underflow is harmless.
    - transposes (PE): x chunks [c,n] -> xT [n,c] via identity matmul (PSUM),
                  copied to SBUF on DVE/ACT.
    - softmax (ACT): numer = Exp(logits + bias) in bf16 (only the aggregation
                  weights pass through bf16; xT stays at f32r precision).
                  numer^T transposes (PE) -> per-n denominator via the
                  accum_out of the PSUM->SBUF copy (DVE tensor_scalar).
    - mm2 (PE):   enc[k,c] = sum_n w^T[n,k]^T @ xT[n,c], weights scaled by
                  1/denom on DVE; wsum via a -1s rhs column; final
                  enc += wsum*cw on DVE (scalar_tensor_tensor).
Matmuls run in float32r (full-rate 4-byte mode, ~1e-4 relative rounding).
End-to-end: ~99 us on hardware, relative error ~1.5e-4 vs the fp32 reference.
"""
import os
import numpy as np

B, C, N, K = 16, 512, 4096, 32
NCORES = 8
BPC = B // NCORES          # batches per core
SEG = 512                  # n per segment
NSEG = N // SEG
CC = C // 128              # c chunks
NB = SEG // 128            # n chunks per segment

_CACHE = {}


def _patch_tile_drain(tile, mybir, ScopedClock):
    """This walrus build rejects any instruction carrying >1 sync wait.
    Split extra waits onto single-wait NoOps on the same engine."""
    if getattr(tile.TileContext, "_multiwait_patched", False):
        return
    tile.TileContext._multiwait_patched = True

    _orig_add = tile.TileContext._add_instruction

    def _split_add(self, inst):
        si = inst.sync_info
        if si is not None and si.on_wait and len(si.on_wait) > 1:
            waits = list(si.on_wait)
            for w in waits[:-1]:
                nop = mybir.InstNoOp(name=f"waitnop-{self.nc.next_id()}", ins=[], outs=[])
                nop.engine = inst.engine
                nop.sync_info = mybir.SyncInfo(on_wait=[w], on_update=[])
                _orig_add(self, nop)
            si.on_wait = [waits[-1]]
            inst.sync_info = si
        _orig_add(self, inst)

    tile.TileContext._add_instruction = _split_add

    def _patched_drain(self, tick_clock, wait_clock):
        nc = self.nc
        probe = nc.sync.drain()
        wait_clock.add_sem_waits(probe.ins, ScopedClock({None: tick_clock.global_clock}))
        raw = probe.ins
        waits = list(raw.sync_info.on_wait) if raw.sync_info and raw.sync_info.on_wait else []
        if raw.sync_info is not None:
            raw.sync_info.on_wait = []
        for w in waits:
            wi = nc.sync.nop()
            wi.ins.sync_info = mybir.SyncInfo(on_wait=[w], on_update=[])
        nc.all_engine_barrier()
        assert self.sems is not None
        popped = nc._tile_sem_poison_stack.pop()
        assert popped is self._sem_poison
        nc.clear_and_free_semaphores(list(self.sems.allocated().values()))
        nc.all_engine_barrier()

    tile.TileContext._drain_and_barrier = _patched_drain


def _build():
    import concourse.bass as bass
    import concourse.tile as tile
    from concourse import mybir
    from concourse.vector_clock import ScopedClock

    _patch_tile_drain(tile, mybir, ScopedClock)

    F32 = mybir.dt.float32
    F32R = mybir.dt.float32r
    BF16 = mybir.dt.bfloat16
    Alu = mybir.AluOpType
    Act = mybir.ActivationFunctionType

    nc = bass.Bass("TRN2", target_bir_lowering=False, debug=False, num_devices=NCORES)
    x_ext = nc.dram_tensor("x", [BPC, C, N], F32R, kind="ExternalInput").ap()
    at_ext = nc.dram_tensor("at", [128, CC, K], F32R, kind="ExternalInput").ap()
    sbc_ext = nc.dram_tensor("sbc", [128, K], F32R, kind="ExternalInput").ap()
    bias_ext = nc.dram_tensor("bias", [K, 1], F32, kind="ExternalInput").ap()
    cw_ext = nc.dram_tensor("cw", [K, C], F32, kind="ExternalInput").ap()
    id_ext = nc.dram_tensor("ident", [128, 128], F32R, kind="ExternalInput").ap()
    id32_ext = nc.dram_tensor("ident32", [K, K], BF16, kind="ExternalInput").ap()
    enc_ext = nc.dram_tensor("enc", [BPC, K, C], F32, kind="ExternalOutput").ap()

    with tile.TileContext(nc) as tc:
        with (
            tc.tile_pool(name="singles", bufs=1) as singles,
            tc.tile_pool(name="xin", bufs=4) as xin,
            tc.tile_pool(name="xts", bufs=8) as xts,
            tc.tile_pool(name="small", bufs=2) as small,
            tc.tile_pool(name="outp", bufs=2) as outp,
            tc.tile_pool(name="ps_xt", bufs=4, space="PSUM") as ps_xt,
            tc.tile_pool(name="ps_lg", bufs=1, space="PSUM") as ps_lg,
            tc.tile_pool(name="ps_nt", bufs=1, space="PSUM") as ps_nt,
            tc.tile_pool(name="ps_enc", bufs=1, space="PSUM") as ps_enc,
            tc.tile_pool(name="ps_ws", bufs=1, space="PSUM") as ps_ws,
        ):
            id_sb = singles.tile([128, 128], F32R)
            nc.gpsimd.dma_start(out=id_sb, in_=id_ext)
            at_sb = singles.tile([128, CC, K], F32R)
            nc.gpsimd.dma_start(out=at_sb, in_=at_ext)
            sbc_sb = singles.tile([128, K], F32R)
            nc.gpsimd.dma_start(out=sbc_sb, in_=sbc_ext)
            bias_sb = singles.tile([K, 1], F32)
            nc.gpsimd.dma_start(out=bias_sb, in_=bias_ext)
            id32_sb = singles.tile([K, K], BF16)
            nc.gpsimd.dma_start(out=id32_sb, in_=id32_ext)
            cw_sb = singles.tile([K, C], F32)
            nc.gpsimd.dma_start(out=cw_sb, in_=cw_ext)
            negones_f = singles.tile([128, 8], F32)
            nc.vector.memset(negones_f, -1.0)
            negones = singles.tile([128, 8], F32R)
            nc.vector.tensor_copy(out=negones, in_=negones_f)

            for b in range(BPC):
                enc_ps = ps_enc.tile([K, C], F32, tag="enc")
                ws_ps = ps_ws.tile([K, 8], F32, tag="ws")
                for s in range(NSEG):
                    first = (s == 0)
                    # ---- load x segment: [p, cc, n512], c = cc*128 + p ----
                    x_sb = xin.tile([128, CC, SEG], F32R, tag="x")
                    nc.sync.dma_start(
                        out=x_sb,
                        in_=x_ext[b].rearrange("(cc p) n -> p cc n", p=128)[
                            :, :, s * SEG:(s + 1) * SEG],
                    )
                    # ---- x^2 in natural layout (one big ACT op); the sum over
                    # c and the scale_k factor ride mm1 on the PE below ----
                    x2sq = xin.tile([128, CC, SEG], F32R, tag="xsq")
                    nc.scalar.activation(
                        out=x2sq.rearrange("p cc n -> p (cc n)"),
                        in_=x_sb.rearrange("p cc n -> p (cc n)"),
                        func=Act.Square)
                    # pair-sum the c-chunks of x^2 on the otherwise-idle GPSIMD
                    # so mm1 only needs 2 x^2 streams instead of 4
                    x2h = xin.tile([128, 2, SEG], F32R, tag="x2h")
                    nc.gpsimd.tensor_add(out=x2h[:, 0, :], in0=x2sq[:, 0, :],
                                         in1=x2sq[:, 1, :])
                    nc.gpsimd.tensor_add(out=x2h[:, 1, :], in0=x2sq[:, 2, :],
                                         in1=x2sq[:, 3, :])
                    # ---- transposes + xT copies (f32r native) ----
                    xt_sbs = []
                    for nb in range(NB):
                        xt_ps = ps_xt.tile([128, C], F32R, tag="xt")
                        for cc in range(CC):
                            nc.tensor.transpose(
                                xt_ps[:, cc * 128:(cc + 1) * 128],
                                x_sb[:, cc, nb * 128:(nb + 1) * 128],
                                id_sb,
                            )
                        xt_sb = xts.tile([128, C], F32R, tag="xts")
                        if nb == 3:
                            nc.scalar.copy(out=xt_sb, in_=xt_ps)
                        else:
                            nc.vector.tensor_copy(out=xt_sb, in_=xt_ps)
                        xt_sbs.append(xt_sb)
                    # ---- mm1: logits^T [K, 512] = AT.T@x + scalebc.T@x^2 ----
                    lg_ps = ps_lg.tile([K, SEG], F32, tag="lg")
                    for cc in range(CC):
                        nc.tensor.matmul(lg_ps, at_sb[:, cc, :], x_sb[:, cc, :],
                                         start=(cc == 0), stop=False)
                    for h in range(2):
                        nc.tensor.matmul(lg_ps, sbc_sb, x2h[:, h, :],
                                         start=False, stop=(h == 1))
                    # ---- softmax numerator (bf16 is fine for the w side) ----
                    numer = small.tile([K, SEG], BF16, tag="numer")
                    nc.scalar.activation(out=numer, in_=lg_ps, func=Act.Exp,
                                         bias=bias_sb, scale=1.0)
                    # ---- numer transposes -> [n,k] + copy-with-denom-accum ----
                    nt_ps = ps_nt.tile([128, NB, K], BF16, tag="nt")
                    for nb in range(NB):
                        nc.tensor.transpose(
                            nt_ps[:, nb, :],
                            numer[:, nb * 128:(nb + 1) * 128],
                            id32_sb)
                    wt_sb = small.tile([128, NB, K], F32R, tag="wt")
                    dcols = small.tile([128, NB], F32, tag="dc")
                    for nb in range(NB):
                        nc.vector.tensor_scalar(
                            out=wt_sb[:, nb, :], in0=nt_ps[:, nb, :],
                            scalar1=1.0, scalar2=0.0,
                            op0=Alu.mult, op1=Alu.add,
                            accum_out=dcols[:, nb:nb + 1])
                    rden = small.tile([128, NB], F32, tag="rd")
                    nc.vector.reciprocal(rden, dcols)
                    for nb in range(NB):
                        nc.vector.tensor_scalar_mul(
                            wt_sb[:, nb, :], in0=wt_sb[:, nb, :],
                            scalar1=rden[:, nb:nb + 1])
                    # ---- mm2 + wsum ----
                    for nb in range(NB):
                        nc.tensor.matmul(enc_ps, wt_sb[:, nb, :], xt_sbs[nb],
                                         start=(first and nb == 0), stop=(s == NSEG - 1 and nb == NB - 1),
                                         skip_group_check=True)
                        nc.tensor.matmul(ws_ps, wt_sb[:, nb, :], negones,
                                         start=(first and nb == 0), stop=(s == NSEG - 1 and nb == NB - 1),
                                         skip_group_check=True)
                # ---- final: enc += wsum * cw (wsum is negative) ----
                enc_sb = outp.tile([K, C], F32, tag="enc_out")
                nc.vector.scalar_tensor_tensor(
                    out=enc_sb, in0=cw_sb, scalar=ws_ps[:, 0:1], in1=enc_ps,
                    op0=Alu.mult, op1=Alu.add)
                nc.sync.dma_start(out=enc_ext[b], in_=enc_sb)

    return nc


def kernel(x, codewords, scale):
    from concourse.bass_utils import run_bass_kernel_spmd

    x = np.ascontiguousarray(x, dtype=np.float32)
    codewords = np.ascontiguousarray(codewords, dtype=np.float32)
    scale = np.ascontiguousarray(scale, dtype=np.float32)

    if "nc" not in _CACHE:
        _CACHE["nc"] = _build()
    nc = _CACHE["nc"]

    # host-side tiny prep
    xr = x.reshape(B, C, N)
    at = (-2.0 * scale[:, None] * codewords).T.copy()          # [C, K]
    at = at.reshape(CC, 128, K).transpose(1, 0, 2).copy()      # [128, cc, K], c = cc*128+p
    c2 = (codewords.astype(np.float64) ** 2).sum(1).astype(np.float32)
    bias = (scale * c2).reshape(K, 1).astype(np.float32)
    sbc = np.broadcast_to(scale.reshape(1, K), (128, K)).astype(np.float32).copy()
    ident = np.eye(128, dtype=np.float32)
    import ml_dtypes
    ident32 = np.eye(K, dtype=ml_dtypes.bfloat16)

    in_maps = []
    for i in range(NCORES):
        in_maps.append({
            "x": np.ascontiguousarray(xr[i * BPC:(i + 1) * BPC]),
            "at": at, "sbc": sbc, "bias": bias,
            "cw": codewords, "ident": ident, "ident32": ident32,
        })
    tmpdir = os.environ.get("BASS_PROF_DIR") or None
    res = run_bass_kernel_spmd(nc, in_maps, list(range(NCORES)), tmpdir=tmpdir)
    _CACHE["last_results"] = res
    out = np.concatenate([res.results[i]["enc"] for i in range(NCORES)], axis=0)
    return out.astype(np.float32)
